# revision 4
# baseline (speedup 1.0000x reference)
"""HypergraphConv (node->edge->node message passing) on 8 Trainium2 NeuronCores.

Computes, for the full (unsharded) inputs:
    xw   = x @ W
    m_e  = (1/deg_e) * sum_{k: edge[k]=e} xw[src[k]]
    o_i  = (1/deg_i) * sum_{k: src[k]=i} m_{edge[k]} + bias
    out  = mean_i relu(o_i)                       # [128]

Sharding/pipeline per core (nodes split 6250/core):
  A: xw for the local node shard (bf16 matmul) -> bf16 row table xwt in DRAM.
  B: phase-1 scatter (node->edge): SWDGE-gather xwt rows per incidence entry
     (entries grouped by edge tile), one-hot-matmul scatter into PSUM,
     partial per-edge sums written bf16 per slab of edge tiles.
  C: per slab: ReduceScatter (bf16 add) across the 8 cores, scale the shard
     by host-computed 1/deg_e, AllGather the bf16 m' table. Slab pipelining
     overlaps the collectives with B's remaining compute.
  D: phase-2 scatter (edge->node): gather m' rows per entry (grouped by node
     tile), one-hot-matmul scatter, scale by host-computed 1/deg_n, +bias,
     relu; the column sum over nodes accumulates in PSUM via a ones-matmul.
  E: column sum -> out_part [128,1]; host sums the 8 cores and divides by N.

Host prep computes degrees (fed as 1/deg tables — no degree matmuls and no
degree collective on device) and sorts/pads both entry streams into static
128-entry chunks shared across cores. One-hot S matrices are built on-device
from compact per-entry target ids via a DVE is_equal against an iota row.
"""

import os
import numpy as np
import ml_dtypes
from contextlib import ExitStack

import concourse.bacc as bacc
import concourse.bass as bass
import concourse.mybir as mybir
import concourse.tile as tile
from concourse import library_config
from concourse.bass_utils import run_bass_kernel_spmd

NCORES = 8
P = 128

N_NODES = 50000
N_EDGES = 20000
IN_DIM = 256
OUT_DIM = 128

BF16 = mybir.dt.bfloat16
F32 = mybir.dt.float32
I16 = mybir.dt.int16

PAD_OH = 200.0  # one-hot id for padding entries: matches no iota column

NSLAB = int(os.environ.get("K2_NSLAB", "2"))
GB1 = int(os.environ.get("K2_GB1", "4"))  # edge tiles per phase-1 gather call
GB2 = int(os.environ.get("K2_GB2", "1"))  # node tiles per phase-2 gather call


def _derived():
    npc = N_NODES // NCORES
    n_node_tiles = (npc + P - 1) // P
    net_real = (N_EDGES + P - 1) // P
    # pad edge tiles so NSLAB slabs are each divisible by the 8-way shard
    step = 8 * NSLAB
    net = ((net_real + step - 1) // step) * step
    return npc, n_node_tiles, net_real, net


def _wrap_idx16(idx):
    """[L] int -> [128, L//16] int16 SWDGE index layout (16-wrap, x8 replicas)."""
    a = np.asarray(idx, dtype=np.int16).reshape(-1, 16).T
    return np.ascontiguousarray(np.tile(a, (8, 1)))


def _oh_cols(oh):
    """[L] float -> [128, L//128] bf16: column c holds entries c*128..c*128+127."""
    return np.ascontiguousarray(oh.reshape(-1, P).T.astype(ml_dtypes.bfloat16))


def _bucket_entries(gidx, tid, n_tiles, chunks, pad_row):
    """Group (gather idx, one-hot id) entry streams by tile with static
    chunk counts shared across cores; pad with (pad_row, PAD_OH)."""
    order = np.argsort(tid, kind="stable")
    gidx = gidx[order]
    tid_s = tid[order]
    counts = np.bincount(tid_s, minlength=n_tiles)
    starts = np.concatenate([[0], np.cumsum(counts[:-1])])
    dest_base = np.concatenate([[0], np.cumsum(chunks[:-1])]) * P
    L = int(chunks.sum()) * P
    g_out = np.full(L, pad_row, dtype=np.int64)
    oh_out = np.full(L, PAD_OH, dtype=np.float32)
    n = gidx.shape[0]
    rank = np.arange(n, dtype=np.int64) - starts[tid_s]
    dest = dest_base[tid_s] + rank
    g_out[dest] = gidx
    return g_out, oh_out, dest, order, L


def build_kernel(chunks1, chunks2, last_nt, bias_is_zero=True):
    npc, n_node_tiles, net_real, net = _derived()
    LA = int(np.sum(chunks1)) * P
    LB = int(np.sum(chunks2)) * P
    NCA = LA // P
    NCB = LB // P
    slab_tiles = net // NSLAB
    shard_tiles = slab_tiles // NCORES
    assert slab_tiles * NSLAB == net and shard_tiles * NCORES == slab_tiles

    nc = bacc.Bacc("TRN2", num_devices=NCORES)

    xT_in = nc.dram_tensor("xT", [IN_DIM, npc], BF16, kind="ExternalInput")
    w_in = nc.dram_tensor("w", [IN_DIM, OUT_DIM], BF16, kind="ExternalInput")
    bias_in = nc.dram_tensor("bias", [1, OUT_DIM], F32, kind="ExternalInput")
    # per-core: column (s*shard_tiles + lt) = 1/deg_e of this core's shard tile
    binv_in = nc.dram_tensor("binv", [P, net // NCORES], F32, kind="ExternalInput")
    dinv_in = nc.dram_tensor("dinv", [P, n_node_tiles], F32, kind="ExternalInput")
    idxA_in = nc.dram_tensor("idxA", [P, LA // 16], I16, kind="ExternalInput")
    ohA_in = nc.dram_tensor("ohA", [P, NCA], BF16, kind="ExternalInput")
    idxB_in = nc.dram_tensor("idxB", [P, LB // 16], I16, kind="ExternalInput")
    ohB_in = nc.dram_tensor("ohB", [P, NCB], BF16, kind="ExternalInput")
    out_part = nc.dram_tensor("out_part", [P, 1], F32, kind="ExternalOutput")

    xwt = nc.dram_tensor("xwt", [npc + P, OUT_DIM], BF16)  # zero row at npc
    mtab = nc.dram_tensor("mtab", [net * P, OUT_DIM], BF16, addr_space="Shared")
    m_part = [nc.dram_tensor(f"mp{s}", [slab_tiles * P, OUT_DIM], BF16)
              for s in range(NSLAB)]
    m_mid = [nc.dram_tensor(f"mm{s}", [shard_tiles * P, OUT_DIM], BF16)
             for s in range(NSLAB)]
    m_shard = [nc.dram_tensor(f"ms{s}", [shard_tiles * P, OUT_DIM], BF16)
               for s in range(NSLAB)]
    rg = [list(range(NCORES))]

    with tile.TileContext(nc) as tc, ExitStack() as ctx:
        pin = ctx.enter_context(tc.tile_pool(name="pin", bufs=1))
        ps_col = ctx.enter_context(tc.tile_pool(name="pscol", bufs=1, space="PSUM"))

        nc.gpsimd.load_library(library_config.mlp)

        iota_i = pin.tile([P, P], I16)
        iota_bf = pin.tile([P, P], BF16)
        nc.gpsimd.iota(iota_i[:], [[1, P]], channel_multiplier=0)
        nc.vector.tensor_copy(out=iota_bf[:], in_=iota_i[:])
        ones_f32 = pin.tile([P, 1], F32)
        nc.vector.memset(ones_f32[:], 1.0)
        bias_bc = pin.tile([P, OUT_DIM], F32)
        nc.sync.dma_start(out=bias_bc[:], in_=bass.AP(bias_in, 0, [[0, P], [1, OUT_DIM]]))
        binv_sb = pin.tile([P, net // NCORES], F32)
        nc.sync.dma_start(out=binv_sb[:], in_=binv_in[:])
        dinv_sb = pin.tile([P, n_node_tiles], F32)
        nc.sync.dma_start(out=dinv_sb[:], in_=dinv_in[:])
        idxA = pin.tile([P, LA // 16], I16)
        ohA = pin.tile([P, NCA], BF16)
        idxB = pin.tile([P, LB // 16], I16)
        ohB = pin.tile([P, NCB], BF16)
        nc.sync.dma_start(out=idxA[:], in_=idxA_in[:])
        nc.sync.dma_start(out=ohA[:], in_=ohA_in[:])
        nc.sync.dma_start(out=idxB[:], in_=idxB_in[:])
        nc.sync.dma_start(out=ohB[:], in_=ohB_in[:])

        pcol = ps_col.tile([P, 1], F32)  # stage-E column accumulator

        def s_build(S_tile, oh_tile, col0, k):
            """S[p, c*128+j] = (oh[p, col0+c] == j), one DVE op for k chunks."""
            s_ap = S_tile[:, :k * P].rearrange("p (k j) -> p k j", k=k)
            o = oh_tile[:, col0:col0 + k]
            in0 = bass.AP(o.tensor, o.offset, [list(o.ap[0]), list(o.ap[1]), [0, P]])
            it = iota_bf[:]
            in1 = bass.AP(it.tensor, it.offset, [list(it.ap[0]), [0, k], [1, P]])
            nc.vector.tensor_tensor(out=s_ap, in0=in0, in1=in1, op=mybir.AluOpType.is_equal)

        # ---- stage A: xw table ------------------------------------------
        with tc.tile_pool(name="pa", bufs=1) as pa, \
             tc.tile_pool(name="pa2", bufs=3) as pa2, \
             tc.tile_pool(name="psa", bufs=2, space="PSUM") as psa:
            kh = IN_DIM // P
            xT_sb = [pa.tile([P, npc], BF16, tag=f"xT{k}", name=f"xT{k}") for k in range(kh)]
            w_sb = [pa.tile([P, OUT_DIM], BF16, tag=f"w{k}", name=f"wsb{k}") for k in range(kh)]
            for k in range(kh):
                nc.sync.dma_start(out=xT_sb[k][:], in_=xT_in[k * P:(k + 1) * P, :])
                nc.sync.dma_start(out=w_sb[k][:], in_=w_in[k * P:(k + 1) * P, :])
            zrow = pa.tile([P, OUT_DIM], BF16)
            nc.vector.memset(zrow[:], 0.0)
            nc.sync.dma_start(out=xwt[npc:npc + P, :], in_=zrow[:])
            for i in range(0, npc, P):
                nt = min(P, npc - i)
                pxw = psa.tile([P, OUT_DIM], F32, tag="pxw")
                for k in range(kh):
                    nc.tensor.matmul(
                        out=pxw[:nt], lhsT=xT_sb[k][:, i:i + nt], rhs=w_sb[k][:],
                        start=(k == 0), stop=(k == kh - 1))
                st = pa2.tile([P, OUT_DIM], BF16, tag="xst")
                nc.scalar.activation(out=st[:nt], in_=pxw[:nt],
                                     func=mybir.ActivationFunctionType.Copy)
                nc.sync.dma_start(out=xwt[i:i + nt, :], in_=st[:nt, :])

        # ---- stages B + C (slab-pipelined) ------------------------------
        cbase1 = np.concatenate([[0], np.cumsum(chunks1)]).astype(int)
        with tc.tile_pool(name="pg", bufs=3) as pg, \
             tc.tile_pool(name="pb", bufs=3) as pb, \
             tc.tile_pool(name="pc", bufs=3) as pc, \
             tc.tile_pool(name="psb", bufs=4, space="PSUM") as psb:
            for s in range(NSLAB):
                t_lo, t_hi = s * slab_tiles, (s + 1) * slab_tiles
                t = t_lo
                while t < t_hi:
                    tb = min(GB1, t_hi - t)
                    bt_chunks = int(cbase1[t + tb] - cbase1[t])
                    G = pg.tile([P, bt_chunks, OUT_DIM], BF16, tag="G1")
                    for g0 in range(0, bt_chunks, 48):
                        gk = min(48, bt_chunks - g0)
                        c0 = int(cbase1[t]) + g0
                        nc.gpsimd.dma_gather(
                            G[:, g0:g0 + gk, :], xwt[:, :],
                            idxA[:, c0 * 8:(c0 + gk) * 8],
                            gk * P, gk * P, OUT_DIM, single_packet=False)
                    for tt in range(t, t + tb):
                        kt = int(chunks1[tt])
                        gb = int(cbase1[tt] - cbase1[t])
                        S = pb.tile([P, kt * P], BF16, tag="S1")
                        s_build(S, ohA, int(cbase1[tt]), kt)
                        pm = psb.tile([P, OUT_DIM], F32, tag="pm")
                        for c in range(kt):
                            nc.tensor.matmul(
                                out=pm[:], lhsT=S[:, c * P:(c + 1) * P],
                                rhs=G[:, gb + c, :],
                                start=(c == 0), stop=(c == kt - 1),
                                skip_group_check=True)
                        mt = pb.tile([P, OUT_DIM], BF16, tag="mt")
                        nc.scalar.activation(out=mt[:], in_=pm[:],
                                             func=mybir.ActivationFunctionType.Copy)
                        nc.sync.dma_start(
                            out=m_part[s][(tt - t_lo) * P:(tt - t_lo + 1) * P, :],
                            in_=mt[:])
                    t += tb

                nc.gpsimd.collective_compute(
                    "ReduceScatter", mybir.AluOpType.add, replica_groups=rg,
                    ins=[m_part[s][:, :]], outs=[m_mid[s][:, :]])
                for lt in range(shard_tiles):
                    r1 = pc.tile([P, OUT_DIM], BF16, tag="r1")
                    nc.sync.dma_start(
                        out=r1[:], in_=m_mid[s][lt * P:(lt + 1) * P, :])
                    bcol = s * shard_tiles + lt
                    mo = pc.tile([P, OUT_DIM], BF16, tag="mo")
                    nc.scalar.activation(
                        out=mo[:], in_=r1[:],
                        func=mybir.ActivationFunctionType.Copy,
                        scale=binv_sb[:, bcol:bcol + 1])
                    nc.sync.dma_start(
                        out=m_shard[s][lt * P:(lt + 1) * P, :], in_=mo[:])
                nc.gpsimd.collective_compute(
                    "AllGather", mybir.AluOpType.bypass, replica_groups=rg,
                    ins=[m_shard[s][:, :]],
                    outs=[mtab[s * slab_tiles * P:(s + 1) * slab_tiles * P, :]])

        # ---- stages D + E -----------------------------------------------
        cbase2 = np.concatenate([[0], np.cumsum(chunks2)]).astype(int)
        with tc.tile_pool(name="pg2", bufs=3) as pg2, \
             tc.tile_pool(name="pd", bufs=3) as pd, \
             tc.tile_pool(name="psd", bufs=4, space="PSUM") as psd:
            t = 0
            while t < n_node_tiles:
                tb = min(GB2, n_node_tiles - t)
                bt_chunks = int(cbase2[t + tb] - cbase2[t])
                G = pg2.tile([P, bt_chunks, OUT_DIM], BF16, tag="G2")
                for g0 in range(0, bt_chunks, 48):
                    gk = min(48, bt_chunks - g0)
                    c0 = int(cbase2[t]) + g0
                    nc.gpsimd.dma_gather(
                        G[:, g0:g0 + gk, :], mtab[:, :],
                        idxB[:, c0 * 8:(c0 + gk) * 8],
                        gk * P, gk * P, OUT_DIM, single_packet=False)
                for tt in range(t, t + tb):
                    kt = int(chunks2[tt])
                    gb = int(cbase2[tt] - cbase2[t])
                    nt = last_nt if tt == n_node_tiles - 1 else P
                    S = pd.tile([P, kt * P], BF16, tag="S2")
                    s_build(S, ohB, int(cbase2[tt]), kt)
                    po = psd.tile([P, OUT_DIM], F32, tag="po")
                    for c in range(kt):
                        nc.tensor.matmul(
                            out=po[:], lhsT=S[:, c * P:(c + 1) * P],
                            rhs=G[:, gb + c, :],
                            start=(c == 0), stop=(c == kt - 1),
                            skip_group_check=True)
                    ot = pd.tile([P, OUT_DIM], F32, tag="ot")
                    if bias_is_zero:
                        # fused: relu(po * dinv) on the otherwise-idle ACT engine
                        nc.scalar.activation(
                            out=ot[:nt], in_=po[:nt],
                            func=mybir.ActivationFunctionType.Relu,
                            scale=dinv_sb[:nt, tt:tt + 1])
                    else:
                        nc.vector.tensor_scalar(
                            out=ot[:nt], in0=po[:nt], scalar1=dinv_sb[:nt, tt:tt + 1],
                            scalar2=None, op0=mybir.AluOpType.mult)
                        nc.vector.tensor_tensor(
                            out=ot[:nt], in0=ot[:nt], in1=bias_bc[:nt],
                            op=mybir.AluOpType.add)
                        nc.vector.tensor_scalar(
                            out=ot[:nt], in0=ot[:nt], scalar1=0.0, scalar2=None,
                            op0=mybir.AluOpType.max)
                    nc.tensor.matmul(
                        out=pcol[:OUT_DIM], lhsT=ot[:nt, :], rhs=ones_f32[:nt],
                        start=(tt == 0), stop=(tt == n_node_tiles - 1),
                        skip_group_check=True)
                t += tb
            with tc.tile_pool(name="pe", bufs=1) as pe:
                ocol = pe.tile([P, 1], F32)
                nc.vector.tensor_copy(out=ocol[:OUT_DIM], in_=pcol[:OUT_DIM])
                nc.sync.dma_start(out=out_part[:, :], in_=ocol[:])

    nc.compile()
    return nc


def prepare_inputs(x, w, bias, hyperedge_index):
    """Host-side sharding: degrees, entry bucketing, static chunk structure."""
    npc, n_node_tiles, net_real, net = _derived()
    src = np.asarray(hyperedge_index[0], dtype=np.int64)
    edge = np.asarray(hyperedge_index[1], dtype=np.int64)

    deg_e = np.bincount(edge, minlength=net * P).astype(np.float64)
    binv_full = np.where(deg_e > 0, 1.0 / np.maximum(deg_e, 1), 0.0).astype(np.float32)
    binv_tiles = binv_full.reshape(net, P).T  # [128, net], column per tile
    slab_tiles = net // NSLAB
    shard_tiles = slab_tiles // NCORES
    deg_n = np.bincount(src, minlength=N_NODES).astype(np.float64)
    dinv_full = np.where(deg_n > 0, 1.0 / np.maximum(deg_n, 1), 0.0).astype(np.float32)

    core_of = src // npc
    per_core = []
    for c in range(NCORES):
        sel = core_of == c
        per_core.append((src[sel] - c * npc, edge[sel]))

    cnt1 = np.zeros((NCORES, net), np.int64)
    cnt2 = np.zeros((NCORES, n_node_tiles), np.int64)
    for c, (s_loc, e_glob) in enumerate(per_core):
        cnt1[c] = np.bincount(e_glob // P, minlength=net)
        cnt2[c] = np.bincount(s_loc // P, minlength=n_node_tiles)
    chunks1 = np.maximum(1, -(-cnt1.max(axis=0) // P))
    chunks2 = np.maximum(1, -(-cnt2.max(axis=0) // P))

    in_maps = []
    for c, (s_loc, e_glob) in enumerate(per_core):
        t1 = e_glob // P
        g1, oh1, dest1, order1, LA = _bucket_entries(s_loc, t1, net, chunks1, npc)
        oh1[dest1] = (e_glob % P)[order1].astype(np.float32)
        t2 = s_loc // P
        g2, oh2, dest2, order2, LB = _bucket_entries(
            e_glob, t2, n_node_tiles, chunks2, N_EDGES)
        oh2[dest2] = (s_loc % P)[order2].astype(np.float32)

        xT = np.ascontiguousarray(
            x[c * npc:(c + 1) * npc].T.astype(ml_dtypes.bfloat16))
        dinv_c = dinv_full[c * npc:(c + 1) * npc]
        dinv_pad = np.zeros(n_node_tiles * P, np.float32)
        dinv_pad[:npc] = dinv_c
        my_tiles = [s * slab_tiles + c * shard_tiles + lt
                    for s in range(NSLAB) for lt in range(shard_tiles)]
        in_maps.append({
            "xT": xT,
            "w": np.ascontiguousarray(w.astype(ml_dtypes.bfloat16)),
            "bias": np.ascontiguousarray(bias.astype(np.float32)).reshape(1, -1),
            "binv": np.ascontiguousarray(binv_tiles[:, my_tiles]),
            "dinv": np.ascontiguousarray(dinv_pad.reshape(n_node_tiles, P).T),
            "idxA": _wrap_idx16(g1),
            "ohA": _oh_cols(oh1),
            "idxB": _wrap_idx16(g2),
            "ohB": _oh_cols(oh2),
        })

    last_nt = npc - (n_node_tiles - 1) * P
    return in_maps, chunks1, chunks2, last_nt


def kernel(x_node_features, lin_weight, bias, hyperedge_index):
    in_maps, chunks1, chunks2, last_nt = prepare_inputs(
        x_node_features, lin_weight, bias, hyperedge_index)
    nc = build_kernel(chunks1, chunks2, last_nt,
                      bias_is_zero=bool(np.all(np.asarray(bias) == 0)))
    res = run_bass_kernel_spmd(nc, in_maps, list(range(NCORES)))
    total = np.zeros(OUT_DIM, np.float64)
    for c in range(NCORES):
        total += res.results[c]["out_part"][:OUT_DIM, 0].astype(np.float64)
    return (total / N_NODES).astype(np.float32)


# revision 5
# speedup vs baseline: 1.2712x; 1.2712x over previous
"""HypergraphConv (node->edge->node message passing) on 8 Trainium2 NeuronCores.

Computes, for the full (unsharded) inputs:
    xw   = x @ W
    m_e  = (1/deg_e) * sum_{k: edge[k]=e} xw[src[k]]
    o_i  = (1/deg_i) * sum_{k: src[k]=i} m_{edge[k]} + bias
    out  = mean_i relu(o_i)                       # [128]

Sharding/pipeline per core (nodes split 6250/core):
  A: xw for the local node shard (bf16 matmul) -> bf16 row table xwt in DRAM.
  B: phase-1 scatter (node->edge): SWDGE-gather xwt rows per incidence entry
     (entries grouped by edge tile), one-hot-matmul scatter into PSUM,
     partial per-edge sums written bf16 per slab of edge tiles.
  C: per slab: ReduceScatter (bf16 add) across the 8 cores, scale the shard
     by host-computed 1/deg_e, AllGather the bf16 m' table. Slab pipelining
     overlaps the collectives with B's remaining compute.
  D: phase-2 scatter (edge->node): gather m' rows per entry (grouped by node
     tile), one-hot-matmul scatter, scale by host-computed 1/deg_n, +bias,
     relu; the column sum over nodes accumulates in PSUM via a ones-matmul.
  E: column sum -> out_part [128,1]; host sums the 8 cores and divides by N.

Host prep computes degrees (fed as 1/deg tables — no degree matmuls and no
degree collective on device) and sorts/pads both entry streams into static
128-entry chunks shared across cores. One-hot S matrices are built on-device
from compact per-entry target ids via a DVE is_equal against an iota row.
"""

import os
import numpy as np
import ml_dtypes
from contextlib import ExitStack

import concourse.bacc as bacc
import concourse.bass as bass
import concourse.mybir as mybir
import concourse.tile as tile
from concourse import library_config
from concourse.bass_utils import run_bass_kernel_spmd

NCORES = 8
P = 128

N_NODES = 50000
N_EDGES = 20000
IN_DIM = 256
OUT_DIM = 128

BF16 = mybir.dt.bfloat16
F32 = mybir.dt.float32
I16 = mybir.dt.int16

PAD_OH = 200.0  # one-hot id for padding entries: matches no iota column

NSLAB = int(os.environ.get("K2_NSLAB", "2"))
GB1 = int(os.environ.get("K2_GB1", "4"))  # edge tiles per phase-1 gather call
GB2 = int(os.environ.get("K2_GB2", "1"))  # node tiles per phase-2 gather call


def _derived():
    npc = N_NODES // NCORES
    n_node_tiles = (npc + P - 1) // P
    net_real = (N_EDGES + P - 1) // P
    # pad edge tiles so NSLAB slabs are each divisible by the 8-way shard
    step = 8 * NSLAB
    net = ((net_real + step - 1) // step) * step
    return npc, n_node_tiles, net_real, net


def _wrap_idx16(idx):
    """[L] int -> [128, L//16] int16 SWDGE index layout (16-wrap, x8 replicas)."""
    a = np.asarray(idx, dtype=np.int16).reshape(-1, 16).T
    return np.ascontiguousarray(np.tile(a, (8, 1)))


def _oh_cols(oh):
    """[L] float -> [128, L//128] bf16: column c holds entries c*128..c*128+127."""
    return np.ascontiguousarray(oh.reshape(-1, P).T.astype(ml_dtypes.bfloat16))


def _bucket_entries(gidx, tid, n_tiles, chunks, pad_row):
    """Group (gather idx, one-hot id) entry streams by tile with static
    chunk counts shared across cores; pad with (pad_row, PAD_OH)."""
    order = np.argsort(tid, kind="stable")
    gidx = gidx[order]
    tid_s = tid[order]
    counts = np.bincount(tid_s, minlength=n_tiles)
    starts = np.concatenate([[0], np.cumsum(counts[:-1])])
    dest_base = np.concatenate([[0], np.cumsum(chunks[:-1])]) * P
    L = int(chunks.sum()) * P
    g_out = np.full(L, pad_row, dtype=np.int64)
    oh_out = np.full(L, PAD_OH, dtype=np.float32)
    n = gidx.shape[0]
    rank = np.arange(n, dtype=np.int64) - starts[tid_s]
    dest = dest_base[tid_s] + rank
    g_out[dest] = gidx
    return g_out, oh_out, dest, order, L


def build_kernel(chunks1, chunks2, last_nt, bias_is_zero=True):
    npc, n_node_tiles, net_real, net = _derived()
    LA = int(np.sum(chunks1)) * P
    LB = int(np.sum(chunks2)) * P
    NCA = LA // P
    NCB = LB // P
    slab_tiles = net // NSLAB
    shard_tiles = slab_tiles // NCORES
    assert slab_tiles * NSLAB == net and shard_tiles * NCORES == slab_tiles

    nc = bacc.Bacc("TRN2", num_devices=NCORES)

    xT_in = nc.dram_tensor("xT", [IN_DIM, npc], BF16, kind="ExternalInput")
    w_in = nc.dram_tensor("w", [IN_DIM, OUT_DIM], BF16, kind="ExternalInput")
    bias_in = nc.dram_tensor("bias", [1, OUT_DIM], F32, kind="ExternalInput")
    # per-core: column (s*shard_tiles + lt) = 1/deg_e of this core's shard tile
    binv_in = nc.dram_tensor("binv", [P, net // NCORES], F32, kind="ExternalInput")
    dinv_in = nc.dram_tensor("dinv", [P, n_node_tiles], F32, kind="ExternalInput")
    idxA_in = nc.dram_tensor("idxA", [P, LA // 16], I16, kind="ExternalInput")
    ohA_in = nc.dram_tensor("ohA", [P, NCA], BF16, kind="ExternalInput")
    idxB_in = nc.dram_tensor("idxB", [P, LB // 16], I16, kind="ExternalInput")
    ohB_in = nc.dram_tensor("ohB", [P, NCB], BF16, kind="ExternalInput")
    out_part = nc.dram_tensor("out_part", [P, 1], F32, kind="ExternalOutput")

    xwt = nc.dram_tensor("xwt", [npc + P, OUT_DIM], BF16)  # zero row at npc
    mtab = nc.dram_tensor("mtab", [net * P, OUT_DIM], BF16, addr_space="Shared")
    m_part = [nc.dram_tensor(f"mp{s}", [slab_tiles * P, OUT_DIM], BF16)
              for s in range(NSLAB)]
    m_mid = [nc.dram_tensor(f"mm{s}", [shard_tiles * P, OUT_DIM], BF16)
             for s in range(NSLAB)]
    m_shard = [nc.dram_tensor(f"ms{s}", [shard_tiles * P, OUT_DIM], BF16)
               for s in range(NSLAB)]
    rg = [list(range(NCORES))]

    with tile.TileContext(nc) as tc, ExitStack() as ctx:
        pin = ctx.enter_context(tc.tile_pool(name="pin", bufs=1))
        ps_col = ctx.enter_context(tc.tile_pool(name="pscol", bufs=1, space="PSUM"))

        nc.gpsimd.load_library(library_config.mlp)

        iota_i = pin.tile([P, P], I16)
        iota_bf = pin.tile([P, P], BF16)
        nc.gpsimd.iota(iota_i[:], [[1, P]], channel_multiplier=0)
        nc.vector.tensor_copy(out=iota_bf[:], in_=iota_i[:])
        ones_f32 = pin.tile([P, 1], F32)
        nc.vector.memset(ones_f32[:], 1.0)
        bias_bc = pin.tile([P, OUT_DIM], F32)
        nc.sync.dma_start(out=bias_bc[:], in_=bass.AP(bias_in, 0, [[0, P], [1, OUT_DIM]]))
        binv_sb = pin.tile([P, net // NCORES], F32)
        nc.sync.dma_start(out=binv_sb[:], in_=binv_in[:])
        dinv_sb = pin.tile([P, n_node_tiles], F32)
        nc.sync.dma_start(out=dinv_sb[:], in_=dinv_in[:])
        idxA = pin.tile([P, LA // 16], I16)
        ohA = pin.tile([P, NCA], BF16)
        idxB = pin.tile([P, LB // 16], I16)
        ohB = pin.tile([P, NCB], BF16)
        nc.sync.dma_start(out=idxA[:], in_=idxA_in[:])
        nc.sync.dma_start(out=ohA[:], in_=ohA_in[:])
        nc.sync.dma_start(out=idxB[:], in_=idxB_in[:])
        nc.sync.dma_start(out=ohB[:], in_=ohB_in[:])

        pcol = ps_col.tile([P, 1], F32)  # stage-E column accumulator

        def s_build(S_tile, oh_tile, col0, k):
            """S[p, c*128+j] = (oh[p, col0+c] == j), one DVE op for k chunks."""
            s_ap = S_tile[:, :k * P].rearrange("p (k j) -> p k j", k=k)
            o = oh_tile[:, col0:col0 + k]
            in0 = bass.AP(o.tensor, o.offset, [list(o.ap[0]), list(o.ap[1]), [0, P]])
            it = iota_bf[:]
            in1 = bass.AP(it.tensor, it.offset, [list(it.ap[0]), [0, k], [1, P]])
            nc.vector.tensor_tensor(out=s_ap, in0=in0, in1=in1, op=mybir.AluOpType.is_equal)

        def s_build_split(S_tile, oh_tile, col0, kt):
            """Two-half S build: lets MMs on the first chunks start earlier."""
            if kt <= 2:
                s_build(S_tile, oh_tile, col0, kt)
                return
            h = kt // 2
            s_build(S_tile, oh_tile, col0, h)
            k2 = kt - h
            s_ap = S_tile[:, h * P:kt * P].rearrange("p (k j) -> p k j", k=k2)
            o = oh_tile[:, col0 + h:col0 + kt]
            in0 = bass.AP(o.tensor, o.offset, [list(o.ap[0]), list(o.ap[1]), [0, P]])
            it = iota_bf[:]
            in1 = bass.AP(it.tensor, it.offset, [list(it.ap[0]), [0, k2], [1, P]])
            nc.vector.tensor_tensor(out=s_ap, in0=in0, in1=in1, op=mybir.AluOpType.is_equal)

        # ---- stage A: xw table ------------------------------------------
        with tc.tile_pool(name="pa", bufs=1) as pa, \
             tc.tile_pool(name="pa2", bufs=3) as pa2, \
             tc.tile_pool(name="psa", bufs=2, space="PSUM") as psa:
            kh = IN_DIM // P
            xT_sb = [pa.tile([P, npc], BF16, tag=f"xT{k}", name=f"xT{k}") for k in range(kh)]
            w_sb = [pa.tile([P, OUT_DIM], BF16, tag=f"w{k}", name=f"wsb{k}") for k in range(kh)]
            for k in range(kh):
                nc.sync.dma_start(out=xT_sb[k][:], in_=xT_in[k * P:(k + 1) * P, :])
                nc.sync.dma_start(out=w_sb[k][:], in_=w_in[k * P:(k + 1) * P, :])
            zrow = pa.tile([P, OUT_DIM], BF16)
            nc.vector.memset(zrow[:], 0.0)
            nc.sync.dma_start(out=xwt[npc:npc + P, :], in_=zrow[:])
            for i in range(0, npc, P):
                nt = min(P, npc - i)
                pxw = psa.tile([P, OUT_DIM], F32, tag="pxw")
                for k in range(kh):
                    nc.tensor.matmul(
                        out=pxw[:nt], lhsT=xT_sb[k][:, i:i + nt], rhs=w_sb[k][:],
                        start=(k == 0), stop=(k == kh - 1))
                st = pa2.tile([P, OUT_DIM], BF16, tag="xst")
                nc.scalar.activation(out=st[:nt], in_=pxw[:nt],
                                     func=mybir.ActivationFunctionType.Copy)
                nc.sync.dma_start(out=xwt[i:i + nt, :], in_=st[:nt, :])

        # ---- stages B + C (slab-pipelined) ------------------------------
        cbase1 = np.concatenate([[0], np.cumsum(chunks1)]).astype(int)
        with tc.tile_pool(name="pg", bufs=3) as pg, \
             tc.tile_pool(name="pb", bufs=3) as pb, \
             tc.tile_pool(name="pc", bufs=3) as pc, \
             tc.tile_pool(name="psb", bufs=5, space="PSUM") as psb:
            for s in range(NSLAB):
                t_lo, t_hi = s * slab_tiles, (s + 1) * slab_tiles
                t = t_lo
                while t < t_hi:
                    tb = min(GB1, t_hi - t)
                    bt_chunks = int(cbase1[t + tb] - cbase1[t])
                    G = pg.tile([P, bt_chunks, OUT_DIM], BF16, tag="G1")
                    for g0 in range(0, bt_chunks, 48):
                        gk = min(48, bt_chunks - g0)
                        c0 = int(cbase1[t]) + g0
                        nc.gpsimd.dma_gather(
                            G[:, g0:g0 + gk, :], xwt[:, :],
                            idxA[:, c0 * 8:(c0 + gk) * 8],
                            gk * P, gk * P, OUT_DIM, single_packet=False)
                    for tt in range(t, t + tb):
                        kt = int(chunks1[tt])
                        gb = int(cbase1[tt] - cbase1[t])
                        S = pb.tile([P, kt * P], BF16, tag="S1")
                        s_build_split(S, ohA, int(cbase1[tt]), kt)
                        pm = psb.tile([P, OUT_DIM], F32, tag="pm")
                        for c in range(kt):
                            nc.tensor.matmul(
                                out=pm[:], lhsT=S[:, c * P:(c + 1) * P],
                                rhs=G[:, gb + c, :],
                                start=(c == 0), stop=(c == kt - 1),
                                skip_group_check=True)
                        mt = pb.tile([P, OUT_DIM], BF16, tag="mt")
                        nc.scalar.activation(out=mt[:], in_=pm[:],
                                             func=mybir.ActivationFunctionType.Copy)
                        nc.sync.dma_start(
                            out=m_part[s][(tt - t_lo) * P:(tt - t_lo + 1) * P, :],
                            in_=mt[:])
                    t += tb

                nc.gpsimd.collective_compute(
                    "ReduceScatter", mybir.AluOpType.add, replica_groups=rg,
                    ins=[m_part[s][:, :]], outs=[m_mid[s][:, :]])
                for lt in range(shard_tiles):
                    r1 = pc.tile([P, OUT_DIM], BF16, tag="r1")
                    nc.sync.dma_start(
                        out=r1[:], in_=m_mid[s][lt * P:(lt + 1) * P, :])
                    bcol = s * shard_tiles + lt
                    mo = pc.tile([P, OUT_DIM], BF16, tag="mo")
                    nc.scalar.activation(
                        out=mo[:], in_=r1[:],
                        func=mybir.ActivationFunctionType.Copy,
                        scale=binv_sb[:, bcol:bcol + 1])
                    nc.sync.dma_start(
                        out=m_shard[s][lt * P:(lt + 1) * P, :], in_=mo[:])
                nc.gpsimd.collective_compute(
                    "AllGather", mybir.AluOpType.bypass, replica_groups=rg,
                    ins=[m_shard[s][:, :]],
                    outs=[mtab[s * slab_tiles * P:(s + 1) * slab_tiles * P, :]])

        # ---- stages D + E -----------------------------------------------
        cbase2 = np.concatenate([[0], np.cumsum(chunks2)]).astype(int)
        with tc.tile_pool(name="pg2", bufs=3) as pg2, \
             tc.tile_pool(name="pd", bufs=3) as pd, \
             tc.tile_pool(name="psd", bufs=5, space="PSUM") as psd:
            t = 0
            while t < n_node_tiles:
                tb = min(GB2, n_node_tiles - t)
                bt_chunks = int(cbase2[t + tb] - cbase2[t])
                G = pg2.tile([P, bt_chunks, OUT_DIM], BF16, tag="G2")
                for g0 in range(0, bt_chunks, 48):
                    gk = min(48, bt_chunks - g0)
                    c0 = int(cbase2[t]) + g0
                    nc.gpsimd.dma_gather(
                        G[:, g0:g0 + gk, :], mtab[:, :],
                        idxB[:, c0 * 8:(c0 + gk) * 8],
                        gk * P, gk * P, OUT_DIM, single_packet=False)
                for tt in range(t, t + tb):
                    kt = int(chunks2[tt])
                    gb = int(cbase2[tt] - cbase2[t])
                    nt = last_nt if tt == n_node_tiles - 1 else P
                    S = pd.tile([P, kt * P], BF16, tag="S2")
                    s_build_split(S, ohB, int(cbase2[tt]), kt)
                    po = psd.tile([P, OUT_DIM], F32, tag="po")
                    for c in range(kt):
                        nc.tensor.matmul(
                            out=po[:], lhsT=S[:, c * P:(c + 1) * P],
                            rhs=G[:, gb + c, :],
                            start=(c == 0), stop=(c == kt - 1),
                            skip_group_check=True)
                    ot = pd.tile([P, OUT_DIM], F32, tag="ot")
                    if bias_is_zero:
                        # fused: relu(po * dinv) on the otherwise-idle ACT engine
                        nc.scalar.activation(
                            out=ot[:nt], in_=po[:nt],
                            func=mybir.ActivationFunctionType.Relu,
                            scale=dinv_sb[:nt, tt:tt + 1])
                    else:
                        nc.vector.tensor_scalar(
                            out=ot[:nt], in0=po[:nt], scalar1=dinv_sb[:nt, tt:tt + 1],
                            scalar2=None, op0=mybir.AluOpType.mult)
                        nc.vector.tensor_tensor(
                            out=ot[:nt], in0=ot[:nt], in1=bias_bc[:nt],
                            op=mybir.AluOpType.add)
                        nc.vector.tensor_scalar(
                            out=ot[:nt], in0=ot[:nt], scalar1=0.0, scalar2=None,
                            op0=mybir.AluOpType.max)
                    nc.tensor.matmul(
                        out=pcol[:OUT_DIM], lhsT=ot[:nt, :], rhs=ones_f32[:nt],
                        start=(tt == 0), stop=(tt == n_node_tiles - 1),
                        skip_group_check=True)
                t += tb
            with tc.tile_pool(name="pe", bufs=1) as pe:
                ocol = pe.tile([P, 1], F32)
                nc.vector.tensor_copy(out=ocol[:OUT_DIM], in_=pcol[:OUT_DIM])
                nc.sync.dma_start(out=out_part[:, :], in_=ocol[:])

    nc.compile()
    return nc


def prepare_inputs(x, w, bias, hyperedge_index):
    """Host-side sharding: degrees, entry bucketing, static chunk structure."""
    npc, n_node_tiles, net_real, net = _derived()
    src = np.asarray(hyperedge_index[0], dtype=np.int64)
    edge = np.asarray(hyperedge_index[1], dtype=np.int64)

    deg_e = np.bincount(edge, minlength=net * P).astype(np.float64)
    binv_full = np.where(deg_e > 0, 1.0 / np.maximum(deg_e, 1), 0.0).astype(np.float32)
    binv_tiles = binv_full.reshape(net, P).T  # [128, net], column per tile
    slab_tiles = net // NSLAB
    shard_tiles = slab_tiles // NCORES
    deg_n = np.bincount(src, minlength=N_NODES).astype(np.float64)
    dinv_full = np.where(deg_n > 0, 1.0 / np.maximum(deg_n, 1), 0.0).astype(np.float32)

    core_of = src // npc
    per_core = []
    for c in range(NCORES):
        sel = core_of == c
        per_core.append((src[sel] - c * npc, edge[sel]))

    cnt1 = np.zeros((NCORES, net), np.int64)
    cnt2 = np.zeros((NCORES, n_node_tiles), np.int64)
    for c, (s_loc, e_glob) in enumerate(per_core):
        cnt1[c] = np.bincount(e_glob // P, minlength=net)
        cnt2[c] = np.bincount(s_loc // P, minlength=n_node_tiles)
    chunks1 = np.maximum(1, -(-cnt1.max(axis=0) // P))
    chunks2 = np.maximum(1, -(-cnt2.max(axis=0) // P))

    in_maps = []
    for c, (s_loc, e_glob) in enumerate(per_core):
        t1 = e_glob // P
        g1, oh1, dest1, order1, LA = _bucket_entries(s_loc, t1, net, chunks1, npc)
        oh1[dest1] = (e_glob % P)[order1].astype(np.float32)
        t2 = s_loc // P
        g2, oh2, dest2, order2, LB = _bucket_entries(
            e_glob, t2, n_node_tiles, chunks2, N_EDGES)
        oh2[dest2] = (s_loc % P)[order2].astype(np.float32)

        xT = np.ascontiguousarray(
            x[c * npc:(c + 1) * npc].T.astype(ml_dtypes.bfloat16))
        dinv_c = dinv_full[c * npc:(c + 1) * npc]
        dinv_pad = np.zeros(n_node_tiles * P, np.float32)
        dinv_pad[:npc] = dinv_c
        my_tiles = [s * slab_tiles + c * shard_tiles + lt
                    for s in range(NSLAB) for lt in range(shard_tiles)]
        in_maps.append({
            "xT": xT,
            "w": np.ascontiguousarray(w.astype(ml_dtypes.bfloat16)),
            "bias": np.ascontiguousarray(bias.astype(np.float32)).reshape(1, -1),
            "binv": np.ascontiguousarray(binv_tiles[:, my_tiles]),
            "dinv": np.ascontiguousarray(dinv_pad.reshape(n_node_tiles, P).T),
            "idxA": _wrap_idx16(g1),
            "ohA": _oh_cols(oh1),
            "idxB": _wrap_idx16(g2),
            "ohB": _oh_cols(oh2),
        })

    last_nt = npc - (n_node_tiles - 1) * P
    return in_maps, chunks1, chunks2, last_nt


def kernel(x_node_features, lin_weight, bias, hyperedge_index):
    in_maps, chunks1, chunks2, last_nt = prepare_inputs(
        x_node_features, lin_weight, bias, hyperedge_index)
    nc = build_kernel(chunks1, chunks2, last_nt,
                      bias_is_zero=bool(np.all(np.asarray(bias) == 0)))
    res = run_bass_kernel_spmd(nc, in_maps, list(range(NCORES)))
    total = np.zeros(OUT_DIM, np.float64)
    for c in range(NCORES):
        total += res.results[c]["out_part"][:OUT_DIM, 0].astype(np.float64)
    return (total / N_NODES).astype(np.float32)


# revision 6
# speedup vs baseline: 1.8245x; 1.4353x over previous
"""HypergraphConv (node->edge->node message passing) on 8 Trainium2 NeuronCores.

Computes, for the full (unsharded) inputs:
    xw   = x @ W
    m_e  = (1/deg_e) * sum_{k: edge[k]=e} xw[src[k]]
    o_i  = (1/deg_i) * sum_{k: src[k]=i} m_{edge[k]} + bias
    out  = mean_i relu(o_i)                       # [128]

Sharding/pipeline per core (nodes split 6250/core):
  A: xw for the local node shard (bf16 matmul) -> bf16 row table xwt in DRAM.
  B: phase-1 scatter (node->edge): SWDGE-gather xwt rows per incidence entry
     (entries grouped by edge tile), one-hot-matmul scatter into PSUM,
     partial per-edge sums written bf16 per slab of edge tiles.
  C: per slab: ReduceScatter (bf16 add) across the 8 cores, scale the shard
     by host-computed 1/deg_e, AllGather the bf16 m' table. Slab pipelining
     overlaps the collectives with B's remaining compute.
  D: phase-2 scatter (edge->node): gather m' rows per entry (grouped by node
     tile), one-hot-matmul scatter, scale by host-computed 1/deg_n, +bias,
     relu; the column sum over nodes accumulates in PSUM via a ones-matmul.
  E: column sum -> out_part [128,1]; host sums the 8 cores and divides by N.

Host prep computes degrees (fed as 1/deg tables — no degree matmuls and no
degree collective on device) and sorts/pads both entry streams into static
128-entry chunks shared across cores. One-hot S matrices are built on-device
from compact per-entry target ids via a DVE is_equal against an iota row.
"""

import os
import numpy as np
import ml_dtypes
from contextlib import ExitStack

import concourse.bacc as bacc
import concourse.bass as bass
import concourse.mybir as mybir
import concourse.tile as tile
from concourse import library_config
from concourse.bass_utils import run_bass_kernel_spmd

NCORES = 8
P = 128

N_NODES = 50000
N_EDGES = 20000
IN_DIM = 256
OUT_DIM = 128

BF16 = mybir.dt.bfloat16
F32 = mybir.dt.float32
I16 = mybir.dt.int16

PAD_OH = 200.0  # one-hot id for padding entries: matches no iota column

NSLAB = int(os.environ.get("K2_NSLAB", "2"))
GB1 = int(os.environ.get("K2_GB1", "4"))  # edge tiles per phase-1 gather call
GB2 = int(os.environ.get("K2_GB2", "1"))  # node tiles per phase-2 gather call


def _derived():
    npc = N_NODES // NCORES
    n_node_tiles = (npc + P - 1) // P
    net_real = (N_EDGES + P - 1) // P
    # pad edge tiles so NSLAB slabs are each divisible by the 8-way shard
    step = 8 * NSLAB
    net = ((net_real + step - 1) // step) * step
    return npc, n_node_tiles, net_real, net


def _wrap_idx16(idx):
    """[L] int -> [128, L//16] int16 SWDGE index layout (16-wrap, x8 replicas)."""
    a = np.asarray(idx, dtype=np.int16).reshape(-1, 16).T
    return np.ascontiguousarray(np.tile(a, (8, 1)))


def _oh_cols(oh):
    """[L] float -> [128, L//128] bf16: column c holds entries c*128..c*128+127."""
    return np.ascontiguousarray(oh.reshape(-1, P).T.astype(ml_dtypes.bfloat16))


def _bucket_entries(gidx, tid, n_tiles, chunks, pad_row):
    """Group (gather idx, one-hot id) entry streams by tile with static
    chunk counts shared across cores; pad with (pad_row, PAD_OH)."""
    order = np.argsort(tid, kind="stable")
    gidx = gidx[order]
    tid_s = tid[order]
    counts = np.bincount(tid_s, minlength=n_tiles)
    starts = np.concatenate([[0], np.cumsum(counts[:-1])])
    dest_base = np.concatenate([[0], np.cumsum(chunks[:-1])]) * P
    L = int(chunks.sum()) * P
    g_out = np.full(L, pad_row, dtype=np.int64)
    oh_out = np.full(L, PAD_OH, dtype=np.float32)
    n = gidx.shape[0]
    rank = np.arange(n, dtype=np.int64) - starts[tid_s]
    dest = dest_base[tid_s] + rank
    g_out[dest] = gidx
    return g_out, oh_out, dest, order, L


def build_kernel(chunks1, chunks2, last_nt, bias_is_zero=True):
    npc, n_node_tiles, net_real, net = _derived()
    LA = int(np.sum(chunks1)) * P
    LB = int(np.sum(chunks2)) * P
    NCA = LA // P
    NCB = LB // P
    slab_tiles = net // NSLAB
    shard_tiles = slab_tiles // NCORES
    assert slab_tiles * NSLAB == net and shard_tiles * NCORES == slab_tiles

    nc = bacc.Bacc("TRN2", num_devices=NCORES)

    xT_in = nc.dram_tensor("xT", [IN_DIM, npc], BF16, kind="ExternalInput")
    w_in = nc.dram_tensor("w", [IN_DIM, OUT_DIM], BF16, kind="ExternalInput")
    bias_in = nc.dram_tensor("bias", [1, OUT_DIM], F32, kind="ExternalInput")
    # per-core: column (s*shard_tiles + lt) = 1/deg_e of this core's shard tile
    binv_in = nc.dram_tensor("binv", [P, net // NCORES], F32, kind="ExternalInput")
    dinv_in = nc.dram_tensor("dinv", [P, n_node_tiles], F32, kind="ExternalInput")
    idxA_in = nc.dram_tensor("idxA", [P, LA // 16], I16, kind="ExternalInput")
    ohA_in = nc.dram_tensor("ohA", [P, NCA], BF16, kind="ExternalInput")
    idxB_in = nc.dram_tensor("idxB", [P, LB // 16], I16, kind="ExternalInput")
    ohB_in = nc.dram_tensor("ohB", [P, NCB], BF16, kind="ExternalInput")
    out_part = nc.dram_tensor("out_part", [P, 1], F32, kind="ExternalOutput")

    xwt = nc.dram_tensor("xwt", [npc + P, OUT_DIM], BF16)  # zero row at npc
    mtab = nc.dram_tensor("mtab", [net * P, OUT_DIM], BF16, addr_space="Shared")
    m_part = [nc.dram_tensor(f"mp{s}", [slab_tiles * P, OUT_DIM], BF16)
              for s in range(NSLAB)]
    m_mid = [nc.dram_tensor(f"mm{s}", [shard_tiles * P, OUT_DIM], BF16)
             for s in range(NSLAB)]
    m_shard = [nc.dram_tensor(f"ms{s}", [shard_tiles * P, OUT_DIM], BF16)
               for s in range(NSLAB)]
    rg = [list(range(NCORES))]

    with tile.TileContext(nc) as tc, ExitStack() as ctx:
        pin = ctx.enter_context(tc.tile_pool(name="pin", bufs=1))
        ps_col = ctx.enter_context(tc.tile_pool(name="pscol", bufs=1, space="PSUM"))

        nc.gpsimd.load_library(library_config.mlp)

        iota_i = pin.tile([P, P], I16)
        iota_bf = pin.tile([P, P], BF16)
        nc.gpsimd.iota(iota_i[:], [[1, P]], channel_multiplier=0)
        nc.vector.tensor_copy(out=iota_bf[:], in_=iota_i[:])
        ones_f32 = pin.tile([P, 1], F32)
        nc.vector.memset(ones_f32[:], 1.0)
        bias_bc = pin.tile([P, OUT_DIM], F32)
        nc.sync.dma_start(out=bias_bc[:], in_=bass.AP(bias_in, 0, [[0, P], [1, OUT_DIM]]))
        binv_sb = pin.tile([P, net // NCORES], F32)
        nc.sync.dma_start(out=binv_sb[:], in_=binv_in[:])
        dinv_sb = pin.tile([P, n_node_tiles], F32)
        nc.sync.dma_start(out=dinv_sb[:], in_=dinv_in[:])
        idxA = pin.tile([P, LA // 16], I16)
        ohA = pin.tile([P, NCA], BF16)
        idxB = pin.tile([P, LB // 16], I16)
        ohB = pin.tile([P, NCB], BF16)
        nc.sync.dma_start(out=idxA[:], in_=idxA_in[:])
        nc.sync.dma_start(out=ohA[:], in_=ohA_in[:])
        nc.sync.dma_start(out=idxB[:], in_=idxB_in[:])
        nc.sync.dma_start(out=ohB[:], in_=ohB_in[:])

        pcol = ps_col.tile([P, 1], F32)  # stage-E column accumulator
        acc_sb = pin.tile([P, OUT_DIM], F32)
        nc.vector.memset(acc_sb[:], 0.0)

        def s_build(S_tile, oh_tile, col0, k):
            """S[p, c*128+j] = (oh[p, col0+c] == j), one DVE op for k chunks."""
            s_ap = S_tile[:, :k * P].rearrange("p (k j) -> p k j", k=k)
            o = oh_tile[:, col0:col0 + k]
            in0 = bass.AP(o.tensor, o.offset, [list(o.ap[0]), list(o.ap[1]), [0, P]])
            it = iota_bf[:]
            in1 = bass.AP(it.tensor, it.offset, [list(it.ap[0]), [0, k], [1, P]])
            nc.vector.tensor_tensor(out=s_ap, in0=in0, in1=in1, op=mybir.AluOpType.is_equal)

        def s_build_split(S_tile, oh_tile, col0, kt):
            """Two-half S build: lets MMs on the first chunks start earlier."""
            if kt <= 2:
                s_build(S_tile, oh_tile, col0, kt)
                return
            h = kt // 2
            s_build(S_tile, oh_tile, col0, h)
            k2 = kt - h
            s_ap = S_tile[:, h * P:kt * P].rearrange("p (k j) -> p k j", k=k2)
            o = oh_tile[:, col0 + h:col0 + kt]
            in0 = bass.AP(o.tensor, o.offset, [list(o.ap[0]), list(o.ap[1]), [0, P]])
            it = iota_bf[:]
            in1 = bass.AP(it.tensor, it.offset, [list(it.ap[0]), [0, k2], [1, P]])
            nc.vector.tensor_tensor(out=s_ap, in0=in0, in1=in1, op=mybir.AluOpType.is_equal)

        # ---- stage A: xw table ------------------------------------------
        with tc.tile_pool(name="pa", bufs=1) as pa, \
             tc.tile_pool(name="pa2", bufs=3) as pa2, \
             tc.tile_pool(name="psa", bufs=2, space="PSUM") as psa:
            kh = IN_DIM // P
            xT_sb = [pa.tile([P, npc], BF16, tag=f"xT{k}", name=f"xT{k}") for k in range(kh)]
            w_sb = [pa.tile([P, OUT_DIM], BF16, tag=f"w{k}", name=f"wsb{k}") for k in range(kh)]
            for k in range(kh):
                nc.sync.dma_start(out=xT_sb[k][:], in_=xT_in[k * P:(k + 1) * P, :])
                nc.sync.dma_start(out=w_sb[k][:], in_=w_in[k * P:(k + 1) * P, :])
            zrow = pa.tile([P, OUT_DIM], BF16)
            nc.vector.memset(zrow[:], 0.0)
            nc.sync.dma_start(out=xwt[npc:npc + P, :], in_=zrow[:])
            for i in range(0, npc, P):
                nt = min(P, npc - i)
                pxw = psa.tile([P, OUT_DIM], F32, tag="pxw")
                for k in range(kh):
                    nc.tensor.matmul(
                        out=pxw[:nt], lhsT=xT_sb[k][:, i:i + nt], rhs=w_sb[k][:],
                        start=(k == 0), stop=(k == kh - 1))
                st = pa2.tile([P, OUT_DIM], BF16, tag="xst")
                nc.scalar.activation(out=st[:nt], in_=pxw[:nt],
                                     func=mybir.ActivationFunctionType.Copy)
                nc.sync.dma_start(out=xwt[i:i + nt, :], in_=st[:nt, :])

        # ---- stages B + C (slab-pipelined) ------------------------------
        cbase1 = np.concatenate([[0], np.cumsum(chunks1)]).astype(int)
        with tc.tile_pool(name="pg", bufs=3) as pg, \
             tc.tile_pool(name="pb", bufs=3) as pb, \
             tc.tile_pool(name="pc", bufs=3) as pc, \
             tc.tile_pool(name="psb", bufs=5, space="PSUM") as psb:
            for s in range(NSLAB):
                t_lo, t_hi = s * slab_tiles, (s + 1) * slab_tiles
                t = t_lo
                while t < t_hi:
                    tb = min(GB1, t_hi - t)
                    bt_chunks = int(cbase1[t + tb] - cbase1[t])
                    G = pg.tile([P, bt_chunks, OUT_DIM], BF16, tag="G1")
                    for g0 in range(0, bt_chunks, 48):
                        gk = min(48, bt_chunks - g0)
                        c0 = int(cbase1[t]) + g0
                        nc.gpsimd.dma_gather(
                            G[:, g0:g0 + gk, :], xwt[:, :],
                            idxA[:, c0 * 8:(c0 + gk) * 8],
                            gk * P, gk * P, OUT_DIM, single_packet=False)
                    for tt in range(t, t + tb):
                        kt = int(chunks1[tt])
                        gb = int(cbase1[tt] - cbase1[t])
                        S = pb.tile([P, kt * P], BF16, tag="S1")
                        s_build_split(S, ohA, int(cbase1[tt]), kt)
                        pm = psb.tile([P, OUT_DIM], F32, tag="pm")
                        for c in range(kt):
                            nc.tensor.matmul(
                                out=pm[:], lhsT=S[:, c * P:(c + 1) * P],
                                rhs=G[:, gb + c, :],
                                start=(c == 0), stop=(c == kt - 1),
                                skip_group_check=True)
                        mt = pb.tile([P, OUT_DIM], BF16, tag="mt")
                        nc.scalar.activation(out=mt[:], in_=pm[:],
                                             func=mybir.ActivationFunctionType.Copy)
                        nc.sync.dma_start(
                            out=m_part[s][(tt - t_lo) * P:(tt - t_lo + 1) * P, :],
                            in_=mt[:])
                    t += tb

                nc.gpsimd.collective_compute(
                    "ReduceScatter", mybir.AluOpType.add, replica_groups=rg,
                    ins=[m_part[s][:, :]], outs=[m_mid[s][:, :]])
                for lt in range(shard_tiles):
                    r1 = pc.tile([P, OUT_DIM], BF16, tag="r1")
                    nc.sync.dma_start(
                        out=r1[:], in_=m_mid[s][lt * P:(lt + 1) * P, :])
                    bcol = s * shard_tiles + lt
                    mo = pc.tile([P, OUT_DIM], BF16, tag="mo")
                    nc.scalar.activation(
                        out=mo[:], in_=r1[:],
                        func=mybir.ActivationFunctionType.Copy,
                        scale=binv_sb[:, bcol:bcol + 1])
                    nc.sync.dma_start(
                        out=m_shard[s][lt * P:(lt + 1) * P, :], in_=mo[:])
                nc.gpsimd.collective_compute(
                    "AllGather", mybir.AluOpType.bypass, replica_groups=rg,
                    ins=[m_shard[s][:, :]],
                    outs=[mtab[s * slab_tiles * P:(s + 1) * slab_tiles * P, :]])

        # ---- stages D + E -----------------------------------------------
        cbase2 = np.concatenate([[0], np.cumsum(chunks2)]).astype(int)
        with tc.tile_pool(name="pg2", bufs=3) as pg2, \
             tc.tile_pool(name="pd", bufs=3) as pd, \
             tc.tile_pool(name="psd", bufs=5, space="PSUM") as psd:
            t = 0
            while t < n_node_tiles:
                tb = min(GB2, n_node_tiles - t)
                bt_chunks = int(cbase2[t + tb] - cbase2[t])
                G = pg2.tile([P, bt_chunks, OUT_DIM], BF16, tag="G2")
                for g0 in range(0, bt_chunks, 48):
                    gk = min(48, bt_chunks - g0)
                    c0 = int(cbase2[t]) + g0
                    nc.gpsimd.dma_gather(
                        G[:, g0:g0 + gk, :], mtab[:, :],
                        idxB[:, c0 * 8:(c0 + gk) * 8],
                        gk * P, gk * P, OUT_DIM, single_packet=False)
                for tt in range(t, t + tb):
                    kt = int(chunks2[tt])
                    gb = int(cbase2[tt] - cbase2[t])
                    nt = last_nt if tt == n_node_tiles - 1 else P
                    S = pd.tile([P, kt * P], BF16, tag="S2")
                    s_build_split(S, ohB, int(cbase2[tt]), kt)
                    po = psd.tile([P, OUT_DIM], F32, tag="po")
                    for c in range(kt):
                        nc.tensor.matmul(
                            out=po[:], lhsT=S[:, c * P:(c + 1) * P],
                            rhs=G[:, gb + c, :],
                            start=(c == 0), stop=(c == kt - 1),
                            skip_group_check=True)
                    ot = pd.tile([P, OUT_DIM], F32, tag="ot")
                    if bias_is_zero:
                        # fused: relu(po * dinv) on the otherwise-idle ACT engine
                        nc.scalar.activation(
                            out=ot[:nt], in_=po[:nt],
                            func=mybir.ActivationFunctionType.Relu,
                            scale=dinv_sb[:nt, tt:tt + 1])
                    else:
                        nc.vector.tensor_scalar(
                            out=ot[:nt], in0=po[:nt], scalar1=dinv_sb[:nt, tt:tt + 1],
                            scalar2=None, op0=mybir.AluOpType.mult)
                        nc.vector.tensor_tensor(
                            out=ot[:nt], in0=ot[:nt], in1=bias_bc[:nt],
                            op=mybir.AluOpType.add)
                        nc.vector.tensor_scalar(
                            out=ot[:nt], in0=ot[:nt], scalar1=0.0, scalar2=None,
                            op0=mybir.AluOpType.max)
                    nc.vector.tensor_tensor(
                        out=acc_sb[:nt], in0=acc_sb[:nt], in1=ot[:nt],
                        op=mybir.AluOpType.add)
                t += tb
            with tc.tile_pool(name="pe", bufs=1) as pe:
                nc.tensor.matmul(
                    out=pcol[:OUT_DIM], lhsT=acc_sb[:, :], rhs=ones_f32[:],
                    start=True, stop=True, skip_group_check=True)
                ocol = pe.tile([P, 1], F32)
                nc.vector.tensor_copy(out=ocol[:OUT_DIM], in_=pcol[:OUT_DIM])
                nc.sync.dma_start(out=out_part[:, :], in_=ocol[:])

    nc.compile()
    return nc


def prepare_inputs(x, w, bias, hyperedge_index):
    """Host-side sharding: degrees, entry bucketing, static chunk structure."""
    npc, n_node_tiles, net_real, net = _derived()
    src = np.asarray(hyperedge_index[0], dtype=np.int64)
    edge = np.asarray(hyperedge_index[1], dtype=np.int64)

    deg_e = np.bincount(edge, minlength=net * P).astype(np.float64)
    binv_full = np.where(deg_e > 0, 1.0 / np.maximum(deg_e, 1), 0.0).astype(np.float32)
    binv_tiles = binv_full.reshape(net, P).T  # [128, net], column per tile
    slab_tiles = net // NSLAB
    shard_tiles = slab_tiles // NCORES
    deg_n = np.bincount(src, minlength=N_NODES).astype(np.float64)
    dinv_full = np.where(deg_n > 0, 1.0 / np.maximum(deg_n, 1), 0.0).astype(np.float32)

    core_of = src // npc
    per_core = []
    for c in range(NCORES):
        sel = core_of == c
        per_core.append((src[sel] - c * npc, edge[sel]))

    cnt1 = np.zeros((NCORES, net), np.int64)
    cnt2 = np.zeros((NCORES, n_node_tiles), np.int64)
    for c, (s_loc, e_glob) in enumerate(per_core):
        cnt1[c] = np.bincount(e_glob // P, minlength=net)
        cnt2[c] = np.bincount(s_loc // P, minlength=n_node_tiles)
    chunks1 = np.maximum(1, -(-cnt1.max(axis=0) // P))
    chunks2 = np.maximum(1, -(-cnt2.max(axis=0) // P))

    in_maps = []
    for c, (s_loc, e_glob) in enumerate(per_core):
        t1 = e_glob // P
        g1, oh1, dest1, order1, LA = _bucket_entries(s_loc, t1, net, chunks1, npc)
        oh1[dest1] = (e_glob % P)[order1].astype(np.float32)
        t2 = s_loc // P
        g2, oh2, dest2, order2, LB = _bucket_entries(
            e_glob, t2, n_node_tiles, chunks2, N_EDGES)
        oh2[dest2] = (s_loc % P)[order2].astype(np.float32)

        xT = np.ascontiguousarray(
            x[c * npc:(c + 1) * npc].T.astype(ml_dtypes.bfloat16))
        dinv_c = dinv_full[c * npc:(c + 1) * npc]
        dinv_pad = np.zeros(n_node_tiles * P, np.float32)
        dinv_pad[:npc] = dinv_c
        my_tiles = [s * slab_tiles + c * shard_tiles + lt
                    for s in range(NSLAB) for lt in range(shard_tiles)]
        in_maps.append({
            "xT": xT,
            "w": np.ascontiguousarray(w.astype(ml_dtypes.bfloat16)),
            "bias": np.ascontiguousarray(bias.astype(np.float32)).reshape(1, -1),
            "binv": np.ascontiguousarray(binv_tiles[:, my_tiles]),
            "dinv": np.ascontiguousarray(dinv_pad.reshape(n_node_tiles, P).T),
            "idxA": _wrap_idx16(g1),
            "ohA": _oh_cols(oh1),
            "idxB": _wrap_idx16(g2),
            "ohB": _oh_cols(oh2),
        })

    last_nt = npc - (n_node_tiles - 1) * P
    return in_maps, chunks1, chunks2, last_nt


def kernel(x_node_features, lin_weight, bias, hyperedge_index):
    in_maps, chunks1, chunks2, last_nt = prepare_inputs(
        x_node_features, lin_weight, bias, hyperedge_index)
    nc = build_kernel(chunks1, chunks2, last_nt,
                      bias_is_zero=bool(np.all(np.asarray(bias) == 0)))
    res = run_bass_kernel_spmd(nc, in_maps, list(range(NCORES)))
    total = np.zeros(OUT_DIM, np.float64)
    for c in range(NCORES):
        total += res.results[c]["out_part"][:OUT_DIM, 0].astype(np.float64)
    return (total / N_NODES).astype(np.float32)
